# revision 1
# baseline (speedup 1.0000x reference)
"""Trainium2 Bass kernel for AttnBlock++ (GroupNorm + 1x1-conv QKV + dense
attention over 64x64 tokens + 1x1-conv out-proj + residual).

Problem shapes: x [4, 128, 64, 64] f32, four 128x128 NIN weights, GroupNorm(32).

Sharding (8 cores): data-parallel over batch B=4 x query-halves. Core c handles
batch b = c//2 and queries [qh*2048, (qh+1)*2048) with qh = c%2. GroupNorm and
the K/V projections for the batch are recomputed on both cores sharing the
batch (cheap); attention is computed only for the core's query half.

Kernel layout choices:
 - Channels C=128 live on SBUF partitions everywhere.
 - Scores are computed transposed: s^T[m, q] = matmul(lhsT=K[:, m-chunk],
   rhs=Q[:, q-group]), so exp(s^T) lands in SBUF already in the layout the
   output matmul needs as its moving operand (contraction over keys m on
   partitions). No transposes of the 4096x4096 probability matrix.
 - Softmax denominators ride a second accumulating matmul with an all-ones
   [128,128] stationary: psum_s[r, q] = sum_m p[m, q] for every partition r,
   giving the per-query sums replicated across partitions, which is exactly
   the broadcast needed to normalize the [c, q] attention output.
 - exp() skips the max-subtraction: scores have std ~0.05 here, and softmax is
   invariant up to float rounding.
 - Matmul operands are bf16 (fp32 PSUM accumulation); GroupNorm stats,
   softmax normalization, and the residual path stay fp32. The residual
   (|h| ~ 0.02 vs |x| ~ 1) damps attention-path rounding ~50x.
 - GroupNorm needs a cross-partition reduce over each group's 4 channels plus
   a broadcast back; both are done with a DRAM roundtrip using strided /
   partition-replicating access patterns (no PE involvement).
 - Walrus's TensorScalar encoding has a single sync-wait slot, so the kernel
   keeps every tensor_scalar down to at most one non-DVE dependency: all
   per-partition scalar constants are packed into one DMA (then re-sliced by
   DVE copies), and the four weight matrices ride one DMA and are sliced
   directly as matmul stationaries.
"""

import math

import numpy as np
import ml_dtypes

import concourse.bass as bass
import concourse.tile as tile
from concourse import bacc, mybir
from concourse.bass_utils import run_bass_kernel_spmd

C = 128          # channels
HW = 64
N = HW * HW      # 4096 tokens per batch
B = 4
NCORES = 8
QH = N // 2      # queries per core
NGROUPS = 32
GS = C // NGROUPS  # channels per group
EPS = 1e-6
FD = 512         # moving free-dim tile
NQG = QH // FD   # query groups per core
NCH = N // 128   # key chunks
BN_SUB = 512     # bn_stats free-dim limit

F32 = mybir.dt.float32
BF16 = mybir.dt.bfloat16
FP8 = mybir.dt.float8e4
AF = mybir.ActivationFunctionType
ALU = mybir.AluOpType
DROW = mybir.MatmulPerfMode.DoubleRow

# cpack columns
NCONST = 8  # b0 b1 b2 b3 gnsc gnbi eps pad


def _build_program(loop_reps=None):
    # loop_reps: wrap the whole body in a hardware For_i loop — used only by
    # the benchmark harness to measure on-device time via wall-clock slope.
    nc = bacc.Bacc("TRN2", target_bir_lowering=False, debug=False,
                   num_devices=NCORES)

    def din(name, shape, dt=F32):
        return nc.dram_tensor(name, shape, dt, kind="ExternalInput").ap()

    # xf: full batch image, channels-major, with the column-halves swapped
    # host-side for odd cores so THIS core's 2048 query columns are always
    # xf[:, :QH]. Key order only permutes the softmax sum, so results are
    # unchanged; this avoids shipping a separate xq slice.
    xf = din("xf", [C, N])
    wpack = din("wpack", [C, 4 * C], BF16)   # w0|w1|w2|w3, w0 pre-scaled
    cpack = din("cpack", [C, NCONST])        # b0|b1|b2|b3|gnsc|gnbi|eps|0
    gmat = din("gmat", [C, NGROUPS])         # 0.25 * group indicator
    gtmat = din("gtmat", [NGROUPS, C])       # group indicator transposed
    y = nc.dram_tensor("y", [C, QH], F32, kind="ExternalOutput").ap()

    import contextlib

    with tile.TileContext(nc) as tc:
        loop_cm = (tc.For_i(0, loop_reps, 1) if loop_reps
                   else contextlib.nullcontext())
        with (
            loop_cm,
            tc.tile_pool(name="const", bufs=1) as constp,
            tc.tile_pool(name="data", bufs=1) as datap,
            tc.tile_pool(name="small", bufs=1) as smallp,
            tc.tile_pool(name="pexp", bufs=8) as ppool,
            tc.tile_pool(name="work", bufs=3) as workp,
            tc.tile_pool(name="mm", bufs=2, space="PSUM") as mmp,
            tc.tile_pool(name="nin", bufs=2, space="PSUM") as ninp,
            tc.tile_pool(name="acco", bufs=1, space="PSUM") as accop,
            tc.tile_pool(name="accs", bufs=1, space="PSUM") as accsp,
        ):
            # ---- constants -------------------------------------------------
            WP = constp.tile([C, 4 * C], BF16, tag="wp")
            nc.gpsimd.dma_start(out=WP, in_=wpack)

            def wt(i):
                return WP[:, i * C:(i + 1) * C]

            CP = constp.tile([C, NCONST], F32, tag="cp")
            nc.gpsimd.dma_start(out=CP, in_=cpack)
            # re-slice constants through DVE so every later consumer's scalar
            # operand is DVE-produced (single-wait rule)
            bt = []
            for i in range(4):
                t = constp.tile([C, 1], F32, tag=f"b{i}")
                nc.vector.tensor_copy(t, CP[:, i:i + 1])
                bt.append(t)
            gnsct = constp.tile([C, 1], F32, tag="gnsc")
            nc.vector.tensor_copy(gnsct, CP[:, 4:5])
            gnbit = constp.tile([C, 1], F32, tag="gnbi")
            nc.vector.tensor_copy(gnbit, CP[:, 5:6])
            epst = constp.tile([C, 1], F32, tag="eps")
            nc.vector.tensor_copy(epst, CP[:, 6:7])
            ones = constp.tile([C, 2, C], FP8, tag="ones")
            nc.vector.memset(ones, 1.0)
            gm = constp.tile([C, NGROUPS], F32, tag="gm")
            nc.gpsimd.dma_start(out=gm, in_=gmat)
            gtm = constp.tile([NGROUPS, C], F32, tag="gtm")
            nc.gpsimd.dma_start(out=gtm, in_=gtmat)

            # ---- load x (chunked so stats can start early) -----------------
            XF = datap.tile([C, N], F32, tag="xf")
            for j in range(8):
                js = slice(j * (N // 8), (j + 1) * (N // 8))
                nc.sync.dma_start(out=XF[:, js], in_=xf[:, js])
            XQ = XF[:, :QH]

            # ---- GroupNorm stats ------------------------------------------
            # per-partition mean/var over all N columns
            stats = smallp.tile([C, N // BN_SUB, 6], F32, tag="bnstats")
            for j in range(N // BN_SUB):
                nc.vector.bn_stats(out=stats[:, j, :],
                                   in_=XF[:, j * BN_SUB:(j + 1) * BN_SUB])
            mv = smallp.tile([C, 2], F32, tag="mv")
            nc.vector.bn_aggr(out=mv, in_=stats)
            # st = [mean, E[x^2]] per partition
            st = smallp.tile([C, 2], F32, tag="st")
            nc.vector.tensor_copy(st[:, 0:1], mv[:, 0:1])
            nc.vector.tensor_tensor(st[:, 1:2], mv[:, 0:1], mv[:, 0:1],
                                    ALU.mult)
            nc.vector.tensor_tensor(st[:, 1:2], st[:, 1:2], mv[:, 1:2],
                                    ALU.add)
            # cross-partition group reduce + broadcast via two tiny matmuls
            # (gm carries the 1/GS averaging factor)
            pg = ninp.tile([NGROUPS, 2], F32, tag="nin")
            nc.tensor.matmul(pg, lhsT=gm, rhs=st, start=True, stop=True)
            gst = smallp.tile([NGROUPS, 2], F32, tag="gst")
            nc.vector.tensor_copy(gst, pg)
            pb = ninp.tile([C, 2], F32, tag="nin")
            nc.tensor.matmul(pb, lhsT=gtm, rhs=gst, start=True, stop=True)
            # rstd = 1/sqrt(var + eps); a = rstd*gamma; bneg = beta - mean*a
            gmean = smallp.tile([C, 1], F32, tag="gmean")
            nc.vector.tensor_copy(gmean, pb[:, 0:1])
            varg = smallp.tile([C, 1], F32, tag="varg")
            nc.vector.tensor_tensor(varg, gmean, gmean, ALU.mult)
            nc.vector.tensor_tensor(varg, pb[:, 1:2], varg, ALU.subtract)
            # rstd = (var+eps)^-0.5 via exp(-0.5*ln(var+eps)): Ln and Exp
            # share one ACT table set, so the whole kernel needs a single
            # ACT_TABLE_LOAD (Sqrt would force a second set on the GN
            # critical path)
            lnv = smallp.tile([C, 1], F32, tag="lnv")
            nc.scalar.activation(out=lnv, in_=varg, func=AF.Ln, bias=epst,
                                 scale=1.0)
            rstd = smallp.tile([C, 1], F32, tag="rstd")
            nc.scalar.activation(out=rstd, in_=lnv, func=AF.Exp, scale=-0.5)
            a_t = smallp.tile([C, 1], F32, tag="a_t")
            nc.vector.tensor_tensor(a_t, rstd, gnsct, ALU.mult)
            bneg = smallp.tile([C, 1], F32, tag="bneg")
            nc.vector.tensor_tensor(bneg, gmean, a_t, ALU.mult)
            nc.vector.tensor_tensor(bneg, gnbit, bneg, ALU.subtract)

            # ---- normalized activations (bf16) + NIN projections ----------
            # interleaved per 512-column chunk so attention group 0 can start
            # as soon as the first K / VT chunks exist
            H = datap.tile([C, N], BF16, tag="h")
            HQ = datap.tile([C, QH], BF16, tag="hq")
            Q = datap.tile([C, QH], BF16, tag="q")
            K = datap.tile([C, N], BF16, tag="k")
            # V transposed: VT[m, c] = sum_c' H[c', m] * W2[c', c], stored
            # fp8 in DoubleRow pairing [m-part, pair, parity, c]
            # (bias b2 is applied later, after softmax normalization)
            VT = datap.tile([C, NCH // 2, 2, C], FP8, tag="vt")

            def hq_q(j):
                js = slice(j * FD, (j + 1) * FD)
                nc.vector.tensor_scalar(out=HQ[:, js], in0=XQ[:, js],
                                        scalar1=a_t, scalar2=bneg,
                                        op0=ALU.mult, op1=ALU.add)
                pq = ninp.tile([C, FD], F32, tag="nin")
                nc.tensor.matmul(pq, lhsT=wt(0), rhs=HQ[:, js],
                                 start=True, stop=True)
                nc.vector.tensor_scalar_add(out=Q[:, js], in0=pq,
                                            scalar1=bt[0])

            hq_q(0)
            for j in range(N // FD):
                js = slice(j * FD, (j + 1) * FD)
                nc.vector.tensor_scalar(out=H[:, js], in0=XF[:, js],
                                        scalar1=a_t, scalar2=bneg,
                                        op0=ALU.mult, op1=ALU.add)
                pk = ninp.tile([C, FD], F32, tag="nin")
                nc.tensor.matmul(pk, lhsT=wt(1), rhs=H[:, js],
                                 start=True, stop=True)
                nc.vector.tensor_scalar_add(out=K[:, js], in0=pk,
                                            scalar1=bt[1])
                for cp in (2 * j, 2 * j + 1):
                    pv = ninp.tile([C, 2, C], F32, tag="nin")
                    for i in range(2):
                        ch = 2 * cp + i
                        nc.tensor.matmul(pv[:, i, :],
                                         lhsT=H[:, ch * 128:(ch + 1) * 128],
                                         rhs=wt(2), start=True, stop=True)
                    nc.vector.tensor_copy(VT[:, cp, :, :], pv)
            for j in range(1, NQG):
                hq_q(j)

            # ---- attention -------------------------------------------------
            # group-end chains are emitted one group late (software pipeline)
            # so the py matmul never blocks the next group's scores in the
            # in-order PE stream
            def attn_group(g):
                qs = slice(g * FD, (g + 1) * FD)
                po = accop.tile([C, FD], F32, tag="po")
                ps = accsp.tile([C, FD], F32, tag="ps")
                for cp in range(NCH // 2):
                    # two key chunks share one 2-bank PSUM tile and one exp
                    psc = mmp.tile([C, 2, FD], F32, tag="mm")
                    for j in range(2):
                        ch = 2 * cp + j
                        nc.tensor.matmul(psc[:, j, :],
                                         lhsT=K[:, ch * 128:(ch + 1) * 128],
                                         rhs=Q[:, qs], start=True, stop=True)
                    P = ppool.tile([C, 2, FD], FP8, tag="p")
                    nc.scalar.activation(out=P, in_=psc, func=AF.Exp)
                    # fp8 DoubleRow: each matmul contracts both chunks of the
                    # pair (256 keys) at 0.5 cycles/row.
                    # po before ps: the reciprocal's wait on ps then covers
                    # po's PE tick, keeping the normalize TT at one wait
                    nc.tensor.matmul(po, lhsT=VT[:, cp, :, :], rhs=P,
                                     start=(cp == 0), stop=(cp == NCH // 2 - 1),
                                     perf_mode=DROW)
                    nc.tensor.matmul(ps, lhsT=ones, rhs=P,
                                     start=(cp == 0), stop=(cp == NCH // 2 - 1),
                                     perf_mode=DROW)
                return po, ps

            def attn_tail(g, po, ps):
                # two 256-wide halves so the recip->AT->ATB->py->YS chain
                # pipelines; matters mainly for the final group
                HF = FD // 2
                for h in range(2):
                    qs = slice(g * FD + h * HF, g * FD + (h + 1) * HF)
                    hs = slice(h * HF, (h + 1) * HF)
                    R = workp.tile([C, HF], F32, tag="r")
                    nc.vector.reciprocal_approx_fast(out=R, in_=ps[:, hs])
                    AT = workp.tile([C, HF], F32, tag="at")
                    nc.vector.tensor_tensor(AT, po[:, hs], R, ALU.mult)
                    ATB = workp.tile([C, HF], BF16, tag="atb")
                    nc.vector.tensor_scalar_add(out=ATB, in0=AT,
                                                scalar1=bt[2])
                    # x + b3 for the residual, computed while DVE is idle
                    XB = workp.tile([C, HF], F32, tag="xb")
                    nc.vector.tensor_scalar_add(out=XB, in0=XQ[:, qs],
                                                scalar1=bt[3])
                    py = ninp.tile([C, HF], F32, tag="nin")
                    nc.tensor.matmul(py, lhsT=wt(3), rhs=ATB, start=True,
                                     stop=True)
                    YS = workp.tile([C, HF], F32, tag="ys")
                    nc.vector.tensor_tensor(YS, py, XB, ALU.add)
                    nc.sync.dma_start(out=y[:, qs], in_=YS)

            pend = None
            for g in range(NQG):
                po, ps = attn_group(g)
                if pend is not None:
                    attn_tail(g - 1, *pend)
                pend = (po, ps)
            attn_tail(NQG - 1, *pend)

    nc.compile()
    return nc


_PROGRAM = None


def _get_program():
    global _PROGRAM
    if _PROGRAM is None:
        _PROGRAM = _build_program()
    return _PROGRAM


_RUNNER = None


def _get_runner():
    """Build (once) a cached jitted multi-core executor for the program.

    Mirrors concourse.bass2jax.run_bass_via_pjrt's multi-core path, but keeps
    the jitted shard_map so repeat kernel() calls skip the jax re-trace and
    NEFF-cache lookup (~1s of host work per call otherwise).
    """
    global _RUNNER
    if _RUNNER is not None:
        return _RUNNER
    import jax
    from concourse import bass2jax, mybir as _mb

    nc = _get_program()
    bass2jax.install_neuronx_cc_hook()
    assert nc.dbg_addr is None
    partition_name = (nc.partition_id_tensor.name
                      if nc.partition_id_tensor else None)
    in_names, out_names, out_avals = [], [], []
    for alloc in nc.m.functions[0].allocations:
        if not isinstance(alloc, _mb.MemoryLocationSet):
            continue
        name = alloc.memorylocations[0].name
        if alloc.kind == "ExternalInput":
            if name != partition_name:
                in_names.append(name)
        elif alloc.kind == "ExternalOutput":
            shape = tuple(alloc.tensor_shape)
            dtype = _mb.dt.np(alloc.dtype)
            out_avals.append(jax.core.ShapedArray(shape, dtype))
    n_params = len(in_names)
    n_outs = len(out_avals)
    out_names = [a.memorylocations[0].name
                 for a in nc.m.functions[0].allocations
                 if isinstance(a, _mb.MemoryLocationSet)
                 and a.kind == "ExternalOutput"]
    all_names = list(in_names) + list(out_names)
    if partition_name is not None:
        all_names.append(partition_name)

    def _body(*args):
        operands = list(args)
        if partition_name is not None:
            operands.append(bass2jax.partition_id_tensor())
        outs = bass2jax._bass_exec_p.bind(
            *operands,
            out_avals=tuple(out_avals),
            in_names=tuple(all_names),
            out_names=tuple(out_names),
            lowering_input_output_aliases=(),
            sim_require_finite=True,
            sim_require_nnan=True,
            nc=nc,
        )
        return tuple(outs)

    devices = jax.devices()[:NCORES]
    mesh = bass2jax.Mesh(np.asarray(devices), ("core",))
    in_specs = (bass2jax.PartitionSpec("core"),) * (n_params + n_outs)
    out_specs = (bass2jax.PartitionSpec("core"),) * n_outs
    donate = tuple(range(n_params, n_params + n_outs))
    sharded = jax.jit(
        bass2jax.shard_map(_body, mesh=mesh, in_specs=in_specs,
                           out_specs=out_specs, check_rep=False),
        donate_argnums=donate, keep_unused=True,
    )
    _RUNNER = (sharded, in_names, out_names, out_avals)
    return _RUNNER


def _run_cached(in_maps):
    sharded, in_names, out_names, out_avals = _get_runner()
    concat_in = [
        np.concatenate([np.asarray(in_maps[c][nm]) for c in range(NCORES)],
                       axis=0)
        for nm in in_names
    ]
    concat_zeros = [
        np.zeros((NCORES * a.shape[0], *a.shape[1:]), a.dtype)
        for a in out_avals
    ]
    out_arrs = sharded(*concat_in, *concat_zeros)
    return [
        {nm: np.asarray(out_arrs[i]).reshape(NCORES, *out_avals[i].shape)[c]
         for i, nm in enumerate(out_names)}
        for c in range(NCORES)
    ]


def _make_in_maps(x, gn_scale, gn_bias, Ws, bs):
    scale = 1.0 / math.sqrt(C)
    bf = ml_dtypes.bfloat16
    wpack = np.concatenate(
        [np.asarray(Ws[0], np.float32) * scale] +
        [np.asarray(Ws[i], np.float32) for i in (1, 2, 3)], axis=1,
    ).astype(bf)
    cpack = np.zeros((C, NCONST), np.float32)
    cpack[:, 0] = np.asarray(bs[0], np.float32) * scale
    for i in (1, 2, 3):
        cpack[:, i] = np.asarray(bs[i], np.float32)
    cpack[:, 4] = np.asarray(gn_scale, np.float32)
    cpack[:, 5] = np.asarray(gn_bias, np.float32)
    cpack[:, 6] = EPS
    gind = np.zeros((C, NGROUPS), np.float32)
    for c in range(C):
        gind[c, c // GS] = 1.0
    gmat = gind / GS
    gtmat = np.ascontiguousarray(gind.T)

    xr = np.asarray(x, np.float32).reshape(B, C, N)
    in_maps = []
    for core in range(NCORES):
        b, qh = core // 2, core % 2
        xfb = xr[b] if qh == 0 else np.concatenate(
            [xr[b][:, QH:], xr[b][:, :QH]], axis=1)
        in_maps.append({
            "xf": np.ascontiguousarray(xfb),
            "wpack": wpack,
            "cpack": cpack,
            "gmat": gmat,
            "gtmat": gtmat,
        })
    return in_maps


def _assemble(results):
    y = np.empty((B, C, N), np.float32)
    for core in range(NCORES):
        b, qh = core // 2, core % 2
        y[b][:, qh * QH:(qh + 1) * QH] = results[core]["y"]
    return y.reshape(B, C, HW, HW)


def kernel(x, gn_scale, gn_bias, W0, b0, W1, b1, W2, b2, W3, b3,
           _trace=False, _tmpdir=None):
    in_maps = _make_in_maps(x, gn_scale, gn_bias,
                            [W0, W1, W2, W3], [b0, b1, b2, b3])
    if _trace:
        res = run_bass_kernel_spmd(_get_program(), in_maps,
                                   core_ids=list(range(NCORES)),
                                   trace=True, tmpdir=_tmpdir)
        return _assemble(res.results), res
    return _assemble(_run_cached(in_maps))

